# revision 9
# baseline (speedup 1.0000x reference)
"""ContrastiveLanguageLoss Trainium2 kernel (8-core data parallel).

Math (per point i, D=512 features, L=200 anchors, K=8 negatives):
    dots[i,l] = f_i . a_l
    pos_dist  = sqrt(f2 - 2*dots[i,lab_i] + a2[lab_i] + eps)
    neg_dist  = mean_k sqrt(f2 - 2*dots[i,neg_ik] + a2[neg_ik] + eps)
    pos_loss  = relu(pos_dist - 0) = pos_dist
    neg_loss  = relu(0.85 - neg_dist)
    loss      = mean(pos_loss) + mean(neg_loss)

Device pipeline per 128-point tile:
    DMA features -> ACT f2=sum(f^2) (accum) -> DVE/GPSIMD downcast bf16
    -> PE transpose (4x 128x128) -> DVE copy PSUM->SBUF
    -> PE matmuls vs (-2 * anchors^T) (bf16) + K=2 ones-matmul adds a2+eps
    -> ACT sqrt(psum + f2) fused with the PSUM->SBUF copy
    -> GPSIMD indirect_copy gathers 10 slots/point (label, 8 negs, pad)
    -> DVE masked-diagonal extract + neg-sum -> ACT relu tail.
"""

import numpy as np
from contextlib import ExitStack

import concourse.bass as bass
import concourse.bacc as bacc
import concourse.tile as tile
from concourse import mybir
from concourse import bass_utils

N_CORES = 8
N_POINTS = 262144
D = 512
L = 200
NNEG = 8
EPS = 1e-7
NEG_THRESH = 0.85
P = 128
NSLOT = 10  # label + 8 negs + 1 pad
AF = mybir.ActivationFunctionType
OP = mybir.AluOpType
f32 = mybir.dt.float32
bf16 = mybir.dt.bfloat16
i32 = mybir.dt.int32
u16 = mybir.dt.uint16


def build_kernel(npts):
    """One SPMD NeuronCore program for an npts-point shard."""
    T = npts // P  # point-tiles; point id = p*T + t
    assert T % 4 == 0
    nc = bacc.Bacc("TRN2", target_bir_lowering=False, debug=False,
                   enable_asserts=True)

    feats = nc.dram_tensor("features", [npts, D], f32, kind="ExternalInput").ap()
    lab32 = nc.dram_tensor("labels32", [npts], i32, kind="ExternalInput").ap()
    neg32 = nc.dram_tensor("negs32", [npts, NNEG], i32, kind="ExternalInput").ap()
    anch = nc.dram_tensor("anchors", [L, D], f32, kind="ExternalInput").ap()
    o_pos = nc.dram_tensor("pos_loss", [npts], f32, kind="ExternalOutput").ap()
    o_neg = nc.dram_tensor("neg_loss", [npts], f32, kind="ExternalOutput").ap()
    o_par = nc.dram_tensor("partials", [2], f32, kind="ExternalOutput").ap()

    with tile.TileContext(nc) as tc:
        with ExitStack() as ctx:
            _body(ctx, tc, feats, lab32, neg32, anch, o_pos, o_neg, o_par, T)
    nc.compile()
    return nc


def _body(ctx, tc, feats, lab32, neg32, anch, o_pos, o_neg, o_par, T):
    nc = tc.nc
    const = ctx.enter_context(tc.tile_pool(name="const", bufs=1))
    xpool = ctx.enter_context(tc.tile_pool(name="x", bufs=3))
    xbpool = ctx.enter_context(tc.tile_pool(name="xb", bufs=3))
    ftpool = ctx.enter_context(tc.tile_pool(name="ftb", bufs=3))
    dpool = ctx.enter_context(tc.tile_pool(name="dist", bufs=3))
    gpool = ctx.enter_context(tc.tile_pool(name="gath", bufs=2))
    tpool = ctx.enter_context(tc.tile_pool(name="tmp", bufs=2))
    pps = ctx.enter_context(tc.tile_pool(name="ftps", bufs=2, space="PSUM"))
    pps2 = ctx.enter_context(tc.tile_pool(name="dots", bufs=2, space="PSUM"))
    ppre = ctx.enter_context(tc.tile_pool(name="pre", bufs=1, space="PSUM"))

    # ---------------- one-time prep ----------------
    # identities (via v[p,i] = i - p == 0) and the 16-diagonal mask
    idt = const.tile([P, P], i32)
    nc.gpsimd.iota(idt[:], pattern=[[1, P]], base=0, channel_multiplier=-1)
    id_bf = const.tile([P, P], bf16)
    nc.vector.tensor_scalar(id_bf[:], idt[:], 0, None, OP.is_equal)
    id_f32 = const.tile([P, P], f32)
    nc.vector.tensor_scalar(id_f32[:], idt[:], 0, None, OP.is_equal)

    m64i = const.tile([P, 64], i32)
    nc.gpsimd.iota(m64i[:], pattern=[[0, 4], [1, 16]], base=0, channel_multiplier=-1)
    m64a = const.tile([P, 64], i32)
    nc.vector.tensor_scalar(m64a[:], m64i[:], 15, None, OP.bitwise_and)
    mask64 = const.tile([P, 64], f32)
    nc.vector.tensor_scalar(mask64[:], m64a[:], 0, None, OP.is_equal)

    # anchors: aTb = transpose(-2 * A) in bf16, chunked [128, 4*200]
    a0 = const.tile([P, D], f32)
    a1 = const.tile([P, D], f32)
    nc.sync.dma_start(a0[:], anch[0:128, :])
    nc.sync.dma_start(a1[0:72, :], anch[128:200, :])
    a0b = const.tile([P, D], bf16)
    a1b = const.tile([P, D], bf16)
    nc.scalar.activation(a0b[:], a0[:], AF.Copy, bias=0.0, scale=-2.0)
    nc.scalar.activation(a1b[0:72, :], a1[0:72, :], AF.Copy, bias=0.0, scale=-2.0)

    junk = const.tile([P, D], bf16)
    a2c0 = const.tile([P, 1], f32)
    a2c1 = const.tile([P, 1], f32)
    nc.scalar.activation(junk[:], a0[:], AF.Square, accum_out=a2c0[:])
    nc.scalar.activation(junk[0:72, :], a1[0:72, :], AF.Square, accum_out=a2c1[0:72, :])

    aTb = const.tile([P, 4 * L], bf16)
    for c in range(4):
        pre = ppre.tile([P, P], bf16)
        nc.tensor.transpose(pre[:, 0:128], a0b[:, c * 128:(c + 1) * 128], id_bf[:])
        nc.vector.tensor_copy(aTb[:, c * L:c * L + 128], pre[:, 0:128])
        pre2 = ppre.tile([P, P], bf16, tag="pre2")
        nc.tensor.transpose(pre2[:, 0:72], a1b[0:72, c * 128:(c + 1) * 128],
                            id_bf[0:72, 0:72])
        nc.vector.tensor_copy(aTb[:, c * L + 128:(c + 1) * L], pre2[:, 0:72])

    # a2 row (fp32, +eps), then bf16 hi/lo split rows for the K=2 matmul
    a2ps = ppre.tile([1, 256], f32, tag="a2ps")
    nc.tensor.transpose(a2ps[0:1, 0:128], a2c0[:, 0:1], id_f32[:])
    nc.tensor.transpose(a2ps[0:1, 128:200], a2c1[0:72, 0:1], id_f32[0:72, 0:72])
    epsb = const.tile([1, 1], f32)
    nc.vector.memset(epsb[:], float(EPS))
    a2row = const.tile([1, L], f32)
    nc.scalar.activation(a2row[:], a2ps[0:1, 0:200], AF.Identity, bias=epsb[:])
    a2hi = const.tile([1, L], bf16)
    a2lo = const.tile([1, L], bf16)
    nc.vector.tensor_copy(a2hi[:], a2row[:])
    nc.vector.tensor_tensor(a2lo[:], a2row[:], a2hi[:], OP.subtract)
    a2rows2 = const.tile([2, L], bf16)
    nc.sync.dma_start(a2rows2[0:1, :], a2hi[:])
    nc.sync.dma_start(a2rows2[1:2, :], a2lo[:])
    ones2 = const.tile([2, P], bf16)
    nc.vector.memset(ones2[:], 1.0)
    onescol = const.tile([P, 1], f32)
    nc.vector.memset(onescol[:], 1.0)

    # index table: [128, T*10] uint16, row p col t*10+s = selection s of point p*T+t
    labsb = const.tile([P, T], i32)
    nc.sync.dma_start(labsb[:], lab32.rearrange("(p t) -> p t", t=T))
    negsb = const.tile([P, T * NNEG], i32)
    nc.sync.dma_start(negsb[:].rearrange("p (t k) -> p t k", k=NNEG),
                      neg32.rearrange("(p t) k -> p t k", t=T))
    idxu = const.tile([P, T * NSLOT], u16)
    nc.vector.memset(idxu[:], 0)
    idxv = idxu[:].rearrange("p (t s) -> p t s", s=NSLOT)
    nc.vector.tensor_copy(idxv[:, :, 0:1], labsb[:].rearrange("p (t o) -> p t o", o=1))
    nc.vector.tensor_copy(idxv[:, :, 1:1 + NNEG],
                          negsb[:].rearrange("p (t k) -> p t k", k=NNEG))

    # whole-shard accumulators
    f2all = const.tile([P, T], f32)
    posbuf = const.tile([P, T], f32)
    negraw = const.tile([P, T], f32)
    negbuf = const.tile([P, T], f32)

    fr = feats.rearrange("(p t) d -> p t d", t=T)

    # ---------------- main loop ----------------
    for g in range(T // 4):
        x4 = xpool.tile([P, 4 * D], f32)
        nc.sync.dma_start(x4[:].rearrange("p (t d) -> p t d", t=4),
                          fr[:, 4 * g:4 * (g + 1), :])
        g4 = gpool.tile([P, 4 * 16 * NSLOT], f32)
        for u in range(4):
            t = 4 * g + u
            x = x4[:, u * D:(u + 1) * D]
            f2c = f2all[:, t:t + 1]
            # f2 = sum(f^2): mostly ACT, every 8th tile on DVE for balance
            if t % 8 == 0:
                jk = tpool.tile([P, D], bf16, tag="jkd")
                nc.vector.scalar_tensor_tensor(jk[:], x, 1.0, x, OP.mult, OP.mult,
                                               accum_out=f2c)
            else:
                jk = tpool.tile([P, D], bf16, tag="jka")
                nc.scalar.activation(jk[:], x, AF.Square, accum_out=f2c)
            # downcast fp32 -> bf16 (DVE / GPSIMD split)
            xb = xbpool.tile([P, D], bf16)
            if t % 10 < 3:
                nc.vector.tensor_copy(xb[:], x)
            else:
                nc.gpsimd.tensor_copy(xb[:], x)
            # transpose 4x [128,128] into one PSUM tile
            ftps = pps.tile([P, D], bf16)
            for c in range(4):
                nc.tensor.transpose(ftps[:, c * 128:(c + 1) * 128],
                                    xb[:, c * 128:(c + 1) * 128], id_bf[:])
            ftb = ftpool.tile([P, D], bf16)
            nc.vector.tensor_copy(ftb[:], ftps[:])
            # dots = fT.T @ (-2 aT)  (+ a2 + eps via K=2 ones matmul)
            ps2 = pps2.tile([P, L], f32)
            for c in range(4):
                nc.tensor.matmul(ps2[:], ftb[:, c * 128:(c + 1) * 128],
                                 aTb[:, c * L:(c + 1) * L],
                                 start=(c == 0), stop=False)
            nc.tensor.matmul(ps2[:], ones2[:], a2rows2[:], start=False, stop=True)
            # dist = sqrt(psum + f2) fused with PSUM->SBUF copy
            dist = dpool.tile([P, L], f32)
            nc.scalar.activation(dist[:], ps2[:], AF.Sqrt, bias=f2c, scale=1.0)
            # gather the 10 slots per point
            nc.gpsimd.indirect_copy(g4[:, u * 160:(u + 1) * 160], dist[:],
                                    idxu[:, t * NSLOT:(t + 1) * NSLOT], True)
        # extraction for the 4 tiles: g4 layout [t=4][s=10][r=16],
        # value for point (p, t) at r == p%16
        gv = g4[:].rearrange("p (t s r) -> p t r s", t=4, s=NSLOT, r=16)
        ns4 = tpool.tile([P, 64], f32, tag="ns4")
        nc.vector.tensor_reduce(ns4[:], gv[:, :, :, 1:1 + NNEG],
                                mybir.AxisListType.X, OP.add)
        pm = tpool.tile([P, 64], f32, tag="pm")
        nc.vector.tensor_tensor(pm[:], gv[:, :, :, 0], mask64[:], OP.mult)
        nc.vector.tensor_reduce(posbuf[:, 4 * g:4 * (g + 1)],
                                pm[:].rearrange("p (t r) -> p t r", r=16),
                                mybir.AxisListType.X, OP.add)
        nm = tpool.tile([P, 64], f32, tag="nm")
        nc.vector.tensor_tensor(nm[:], ns4[:], mask64[:], OP.mult)
        nc.vector.tensor_reduce(negraw[:, 4 * g:4 * (g + 1)],
                                nm[:].rearrange("p (t r) -> p t r", r=16),
                                mybir.AxisListType.X, OP.add)

    # ---------------- epilogue ----------------
    thrb = const.tile([P, 1], f32)
    nc.vector.memset(thrb[:], float(NEG_THRESH))
    for c0 in range(0, T, 64):
        w = min(64, T - c0)
        nc.scalar.activation(negbuf[:, c0:c0 + w], negraw[:, c0:c0 + w],
                             AF.Relu, bias=thrb[:], scale=-1.0 / NNEG)
    nc.sync.dma_start(o_pos.rearrange("(p t) -> p t", t=T), posbuf[:])
    nc.sync.dma_start(o_neg.rearrange("(p t) -> p t", t=T), negbuf[:])

    sums = const.tile([P, 2], f32)
    nc.vector.tensor_reduce(sums[:, 0:1], posbuf[:], mybir.AxisListType.X, OP.add)
    nc.vector.tensor_reduce(sums[:, 1:2], negbuf[:], mybir.AxisListType.X, OP.add)
    parps = ppre.tile([2, 1], f32, tag="parps")
    nc.tensor.matmul(parps[:], sums[:], onescol[:], start=True, stop=True)
    parsb = const.tile([2, 1], f32)
    nc.scalar.copy(parsb[:], parps[:])
    nc.sync.dma_start(o_par[:].rearrange("(p o) -> p o", o=1), parsb[:])


_NC_CACHE = {}


def _get_nc(npts):
    if npts not in _NC_CACHE:
        _NC_CACHE[npts] = build_kernel(npts)
    return _NC_CACHE[npts]


def run_sharded(features, labels, anchor_feats, neg_inds, **spmd_kwargs):
    features = np.ascontiguousarray(np.asarray(features), dtype=np.float32)
    anchor_feats = np.ascontiguousarray(np.asarray(anchor_feats), dtype=np.float32)
    labels = np.ascontiguousarray(np.asarray(labels))
    neg_inds = np.ascontiguousarray(np.asarray(neg_inds))
    n = features.shape[0]
    shard = n // N_CORES
    lab_v = np.ascontiguousarray(labels.astype(np.int32, copy=False)).reshape(n)
    neg_v = np.ascontiguousarray(neg_inds.astype(np.int32, copy=False)).reshape(n, NNEG)

    nc = _get_nc(shard)
    in_maps = []
    for c in range(N_CORES):
        sl = slice(c * shard, (c + 1) * shard)
        in_maps.append({
            "features": features[sl],
            "labels32": np.ascontiguousarray(lab_v[sl]),
            "negs32": np.ascontiguousarray(neg_v[sl]),
            "anchors": anchor_feats,
        })
    res = bass_utils.run_bass_kernel_spmd(nc, in_maps, list(range(N_CORES)),
                                          **spmd_kwargs)
    outs = res.results
    pos = np.concatenate([outs[c]["pos_loss"] for c in range(N_CORES)])
    neg = np.concatenate([outs[c]["neg_loss"] for c in range(N_CORES)])
    tot = np.sum([outs[c]["partials"].astype(np.float64) for c in range(N_CORES)])
    loss = np.float32(tot / n)
    return (loss, pos, neg), res


def kernel(features, labels, anchor_feats, neg_inds):
    out, _ = run_sharded(features, labels, anchor_feats, neg_inds)
    return out


# revision 12
# speedup vs baseline: 1.0693x; 1.0693x over previous
"""ContrastiveLanguageLoss Trainium2 kernel (8-core data parallel).

Math (per point i, D=512 features, L=200 anchors, K=8 negatives):
    dots[i,l] = f_i . a_l
    pos_dist  = sqrt(f2 - 2*dots[i,lab_i] + a2[lab_i] + eps)
    neg_dist  = mean_k sqrt(f2 - 2*dots[i,neg_ik] + a2[neg_ik] + eps)
    pos_loss  = relu(pos_dist - 0) = pos_dist
    neg_loss  = relu(0.85 - neg_dist)
    loss      = mean(pos_loss) + mean(neg_loss)

Device pipeline per 128-point tile:
    DMA features -> ACT f2=sum(f^2) (accum) -> DVE/GPSIMD downcast bf16
    -> PE transpose (4x 128x128) -> DVE copy PSUM->SBUF
    -> PE matmuls vs (-2 * anchors^T) (bf16) + K=2 ones-matmul adds a2+eps
    -> ACT sqrt(psum + f2) fused with the PSUM->SBUF copy
    -> GPSIMD indirect_copy gathers 10 slots/point (label, 8 negs, pad)
    -> DVE masked-diagonal extract + neg-sum -> ACT relu tail.
"""

import numpy as np
from contextlib import ExitStack

import concourse.bass as bass
import concourse.bacc as bacc
import concourse.tile as tile
from concourse import mybir
from concourse import bass_utils

N_CORES = 8
N_POINTS = 262144
D = 512
L = 200
NNEG = 8
EPS = 1e-7
NEG_THRESH = 0.85
P = 128
NSLOT = 10  # label + 8 negs + 1 pad
AF = mybir.ActivationFunctionType
OP = mybir.AluOpType
f32 = mybir.dt.float32
bf16 = mybir.dt.bfloat16
i32 = mybir.dt.int32
u16 = mybir.dt.uint16


def build_kernel(npts, repeats=1):
    """One SPMD NeuronCore program for an npts-point shard."""
    T = npts // P  # point-tiles; point id = p*T + t
    assert T % 4 == 0
    nc = bacc.Bacc("TRN2", target_bir_lowering=False, debug=False,
                   enable_asserts=True)

    feats = nc.dram_tensor("features", [npts, D], f32, kind="ExternalInput").ap()
    lab32 = nc.dram_tensor("labels32", [npts], i32, kind="ExternalInput").ap()
    neg32 = nc.dram_tensor("negs32", [npts, NNEG], i32, kind="ExternalInput").ap()
    anch = nc.dram_tensor("anchors", [L, D], f32, kind="ExternalInput").ap()
    o_pos = nc.dram_tensor("pos_loss", [npts], f32, kind="ExternalOutput").ap()
    o_neg = nc.dram_tensor("neg_loss", [npts], f32, kind="ExternalOutput").ap()
    o_par = nc.dram_tensor("partials", [2], f32, kind="ExternalOutput").ap()

    with tile.TileContext(nc) as tc:
        with ExitStack() as ctx:
            _body(ctx, tc, feats, lab32, neg32, anch, o_pos, o_neg, o_par, T,
                  repeats)
    nc.compile()
    return nc


def _body(ctx, tc, feats, lab32, neg32, anch, o_pos, o_neg, o_par, T, repeats=1):
    nc = tc.nc
    const = ctx.enter_context(tc.tile_pool(name="const", bufs=1))
    xpool = ctx.enter_context(tc.tile_pool(name="x", bufs=3))
    xbpool = ctx.enter_context(tc.tile_pool(name="xb", bufs=3))
    ftpool = ctx.enter_context(tc.tile_pool(name="ftb", bufs=3))
    dpool = ctx.enter_context(tc.tile_pool(name="dist", bufs=3))
    gpool = ctx.enter_context(tc.tile_pool(name="gath", bufs=2))
    tpool = ctx.enter_context(tc.tile_pool(name="tmp", bufs=2))
    pps = ctx.enter_context(tc.tile_pool(name="ftps", bufs=2, space="PSUM"))
    pps2 = ctx.enter_context(tc.tile_pool(name="dots", bufs=2, space="PSUM"))
    ppre = ctx.enter_context(tc.tile_pool(name="pre", bufs=1, space="PSUM"))

    # ---------------- one-time prep ----------------
    # identities (via v[p,i] = i - p == 0) and the 16-diagonal mask
    idt = const.tile([P, P], i32)
    nc.gpsimd.iota(idt[:], pattern=[[1, P]], base=0, channel_multiplier=-1)
    id_bf = const.tile([P, P], bf16)
    nc.vector.tensor_scalar(id_bf[:], idt[:], 0, None, OP.is_equal)
    id_f32 = const.tile([P, P], f32)
    nc.vector.tensor_scalar(id_f32[:], idt[:], 0, None, OP.is_equal)

    m64i = const.tile([P, 64], i32)
    nc.gpsimd.iota(m64i[:], pattern=[[0, 4], [1, 16]], base=0, channel_multiplier=-1)
    m64a = const.tile([P, 64], i32)
    nc.vector.tensor_scalar(m64a[:], m64i[:], 15, None, OP.bitwise_and)
    mask64 = const.tile([P, 64], f32)
    nc.vector.tensor_scalar(mask64[:], m64a[:], 0, None, OP.is_equal)

    # anchors: aTb = transpose(-2 * A) in bf16, chunked [128, 4*200]
    a0 = const.tile([P, D], f32)
    a1 = const.tile([P, D], f32)
    nc.sync.dma_start(a0[:], anch[0:128, :])
    nc.sync.dma_start(a1[0:72, :], anch[128:200, :])
    a0b = const.tile([P, D], bf16)
    a1b = const.tile([P, D], bf16)
    nc.scalar.activation(a0b[:], a0[:], AF.Copy, bias=0.0, scale=-2.0)
    nc.scalar.activation(a1b[0:72, :], a1[0:72, :], AF.Copy, bias=0.0, scale=-2.0)

    junk = const.tile([P, D], bf16)
    a2c0 = const.tile([P, 1], f32)
    a2c1 = const.tile([P, 1], f32)
    nc.scalar.activation(junk[:], a0[:], AF.Square, accum_out=a2c0[:])
    nc.scalar.activation(junk[0:72, :], a1[0:72, :], AF.Square, accum_out=a2c1[0:72, :])

    aTb = const.tile([P, 4 * L], bf16)
    for c in range(4):
        pre = ppre.tile([P, P], bf16)
        nc.tensor.transpose(pre[:, 0:128], a0b[:, c * 128:(c + 1) * 128], id_bf[:])
        nc.vector.tensor_copy(aTb[:, c * L:c * L + 128], pre[:, 0:128])
        pre2 = ppre.tile([P, P], bf16, tag="pre2")
        nc.tensor.transpose(pre2[:, 0:72], a1b[0:72, c * 128:(c + 1) * 128],
                            id_bf[0:72, 0:72])
        nc.vector.tensor_copy(aTb[:, c * L + 128:(c + 1) * L], pre2[:, 0:72])

    # a2 row (fp32, +eps), then bf16 hi/lo split rows for the K=2 matmul
    a2ps = ppre.tile([1, 256], f32, tag="a2ps")
    nc.tensor.transpose(a2ps[0:1, 0:128], a2c0[:, 0:1], id_f32[:])
    nc.tensor.transpose(a2ps[0:1, 128:200], a2c1[0:72, 0:1], id_f32[0:72, 0:72])
    epsb = const.tile([1, 1], f32)
    nc.vector.memset(epsb[:], float(EPS))
    a2row = const.tile([1, L], f32)
    nc.scalar.activation(a2row[:], a2ps[0:1, 0:200], AF.Identity, bias=epsb[:])
    a2hi = const.tile([1, L], bf16)
    a2lo = const.tile([1, L], bf16)
    nc.vector.tensor_copy(a2hi[:], a2row[:])
    nc.vector.tensor_tensor(a2lo[:], a2row[:], a2hi[:], OP.subtract)
    a2rows2 = const.tile([2, L], bf16)
    nc.sync.dma_start(a2rows2[0:1, :], a2hi[:])
    nc.sync.dma_start(a2rows2[1:2, :], a2lo[:])
    ones2 = const.tile([2, P], bf16)
    nc.vector.memset(ones2[:], 1.0)
    onescol = const.tile([P, 1], f32)
    nc.vector.memset(onescol[:], 1.0)

    # index table: [128, T*10] uint16, row p col t*10+s = selection s of point p*T+t
    labsb = const.tile([P, T], i32)
    nc.sync.dma_start(labsb[:], lab32.rearrange("(p t) -> p t", t=T))
    negsb = const.tile([P, T * NNEG], i32)
    nc.sync.dma_start(negsb[:].rearrange("p (t k) -> p t k", k=NNEG),
                      neg32.rearrange("(p t) k -> p t k", t=T))
    idxu = const.tile([P, T * NSLOT], u16)
    nc.vector.memset(idxu[:], 0)
    idxv = idxu[:].rearrange("p (t s) -> p t s", s=NSLOT)
    nc.vector.tensor_copy(idxv[:, :, 0:1], labsb[:].rearrange("p (t o) -> p t o", o=1))
    nc.vector.tensor_copy(idxv[:, :, 1:1 + NNEG],
                          negsb[:].rearrange("p (t k) -> p t k", k=NNEG))

    # whole-shard accumulators
    f2all = const.tile([P, T], f32)
    posbuf = const.tile([P, T], f32)
    negraw = const.tile([P, T], f32)
    negbuf = const.tile([P, T], f32)

    fr = feats.rearrange("(p t) d -> p t d", t=T)

    # ---------------- main loop ----------------
    for g in [g for _ in range(repeats) for g in range(T // 4)]:
        x4 = xpool.tile([P, 4 * D], f32)
        nc.sync.dma_start(x4[:].rearrange("p (t d) -> p t d", t=4),
                          fr[:, 4 * g:4 * (g + 1), :])
        g4 = gpool.tile([P, 4 * 16 * NSLOT], f32)
        for u in range(4):
            t = 4 * g + u
            x = x4[:, u * D:(u + 1) * D]
            f2c = f2all[:, t:t + 1]
            # f2 = sum(f^2): mostly ACT, every 8th tile on DVE for balance
            if t % 8 == 0:
                jk = tpool.tile([P, D], bf16, tag="jkd")
                nc.vector.scalar_tensor_tensor(jk[:], x, 1.0, x, OP.mult, OP.mult,
                                               accum_out=f2c)
            else:
                jk = tpool.tile([P, D], bf16, tag="jka")
                nc.scalar.activation(jk[:], x, AF.Square, accum_out=f2c)
            # downcast fp32 -> bf16 (DVE / GPSIMD split)
            xb = xbpool.tile([P, D], bf16)
            if t % 10 < 3:
                nc.vector.tensor_copy(xb[:], x)
            else:
                nc.gpsimd.tensor_copy(xb[:], x)
            # transpose 4x [128,128] into one PSUM tile
            ftps = pps.tile([P, D], bf16)
            for c in range(4):
                nc.tensor.transpose(ftps[:, c * 128:(c + 1) * 128],
                                    xb[:, c * 128:(c + 1) * 128], id_bf[:])
            ftb = ftpool.tile([P, D], bf16)
            nc.vector.tensor_copy(ftb[:], ftps[:])
            # dots = fT.T @ (-2 aT)  (+ a2 + eps via K=2 ones matmul)
            ps2 = pps2.tile([P, L], f32)
            for c in range(4):
                nc.tensor.matmul(ps2[:], ftb[:, c * 128:(c + 1) * 128],
                                 aTb[:, c * L:(c + 1) * L],
                                 start=(c == 0), stop=False)
            nc.tensor.matmul(ps2[:], ones2[:], a2rows2[:], start=False, stop=True)
            # dist = sqrt(psum + f2) fused with PSUM->SBUF copy
            dist = dpool.tile([P, L], f32)
            nc.scalar.activation(dist[:], ps2[:], AF.Sqrt, bias=f2c, scale=1.0)
            # gather the 10 slots per point
            nc.gpsimd.indirect_copy(g4[:, u * 160:(u + 1) * 160], dist[:],
                                    idxu[:, t * NSLOT:(t + 1) * NSLOT], True)
        # extraction for the 4 tiles: g4 layout [t=4][s=10][r=16],
        # value for point (p, t) at r == p%16
        gv = g4[:].rearrange("p (t s r) -> p t r s", t=4, s=NSLOT, r=16)
        ns4 = tpool.tile([P, 64], f32, tag="ns4")
        nc.vector.tensor_reduce(ns4[:], gv[:, :, :, 1:1 + NNEG],
                                mybir.AxisListType.X, OP.add)
        pm = tpool.tile([P, 64], f32, tag="pm")
        nc.vector.tensor_tensor(pm[:], gv[:, :, :, 0], mask64[:], OP.mult)
        nc.vector.tensor_reduce(posbuf[:, 4 * g:4 * (g + 1)],
                                pm[:].rearrange("p (t r) -> p t r", r=16),
                                mybir.AxisListType.X, OP.add)
        nm = tpool.tile([P, 64], f32, tag="nm")
        nc.vector.tensor_tensor(nm[:], ns4[:], mask64[:], OP.mult)
        nc.vector.tensor_reduce(negraw[:, 4 * g:4 * (g + 1)],
                                nm[:].rearrange("p (t r) -> p t r", r=16),
                                mybir.AxisListType.X, OP.add)

    # ---------------- epilogue ----------------
    thrb = const.tile([P, 1], f32)
    nc.vector.memset(thrb[:], float(NEG_THRESH))
    for c0 in range(0, T, 64):
        w = min(64, T - c0)
        nc.scalar.activation(negbuf[:, c0:c0 + w], negraw[:, c0:c0 + w],
                             AF.Relu, bias=thrb[:], scale=-1.0 / NNEG)
    nc.sync.dma_start(o_pos.rearrange("(p t) -> p t", t=T), posbuf[:])
    nc.sync.dma_start(o_neg.rearrange("(p t) -> p t", t=T), negbuf[:])

    sums = const.tile([P, 2], f32)
    nc.vector.tensor_reduce(sums[:, 0:1], posbuf[:], mybir.AxisListType.X, OP.add)
    nc.vector.tensor_reduce(sums[:, 1:2], negbuf[:], mybir.AxisListType.X, OP.add)
    parps = ppre.tile([2, 1], f32, tag="parps")
    nc.tensor.matmul(parps[:], sums[:], onescol[:], start=True, stop=True)
    parsb = const.tile([2, 1], f32)
    nc.scalar.copy(parsb[:], parps[:])
    nc.sync.dma_start(o_par[:].rearrange("(p o) -> p o", o=1), parsb[:])


_NC_CACHE = {}


def _get_nc(npts):
    if npts not in _NC_CACHE:
        _NC_CACHE[npts] = build_kernel(npts)
    return _NC_CACHE[npts]


def run_sharded(features, labels, anchor_feats, neg_inds, **spmd_kwargs):
    features = np.ascontiguousarray(np.asarray(features), dtype=np.float32)
    anchor_feats = np.ascontiguousarray(np.asarray(anchor_feats), dtype=np.float32)
    labels = np.ascontiguousarray(np.asarray(labels))
    neg_inds = np.ascontiguousarray(np.asarray(neg_inds))
    n = features.shape[0]
    shard = n // N_CORES
    lab_v = np.ascontiguousarray(labels.astype(np.int32, copy=False)).reshape(n)
    neg_v = np.ascontiguousarray(neg_inds.astype(np.int32, copy=False)).reshape(n, NNEG)

    nc = _get_nc(shard)
    in_maps = []
    for c in range(N_CORES):
        sl = slice(c * shard, (c + 1) * shard)
        in_maps.append({
            "features": features[sl],
            "labels32": np.ascontiguousarray(lab_v[sl]),
            "negs32": np.ascontiguousarray(neg_v[sl]),
            "anchors": anchor_feats,
        })
    res = bass_utils.run_bass_kernel_spmd(nc, in_maps, list(range(N_CORES)),
                                          **spmd_kwargs)
    outs = res.results
    pos = np.concatenate([outs[c]["pos_loss"] for c in range(N_CORES)])
    neg = np.concatenate([outs[c]["neg_loss"] for c in range(N_CORES)])
    tot = np.sum([outs[c]["partials"].astype(np.float64) for c in range(N_CORES)])
    loss = np.float32(tot / n)
    return (loss, pos, neg), res


def kernel(features, labels, anchor_feats, neg_inds):
    out, _ = run_sharded(features, labels, anchor_feats, neg_inds)
    return out
